# revision 4
# baseline (speedup 1.0000x reference)
"""Trainium2 Bass kernel for nn_CAModel (neural cellular automaton step).

v2 strategy (data-parallel over batch, 16 samples -> 8 cores x 2):
  - The w-direction sobel taps are folded into mm1's contraction dim:
    S rows = [x(16); V(w-1)(16); V(w+1)(16); D(w-1); D(w); D(w+1); ones]
    with V = [1,2,1]_h smoothing and D = x(h+1)-x(h-1).  K=97, same PE
    column count as K=48, so the w-direction conv is free on the PE and
    only 2 big DVE ops (V via STT, D via TT) + 1 Pool op (A) remain.
  - The ones row carries b1 into the matmul, so the relu evacuation is a
    plain max(psum,0) -- legal on both ScalarE (ACTIVATE) and VectorE
    (TENSOR_SCALAR), split by a tunable pattern (PSUM reads are the
    bottleneck: ~1 elem/cycle/partition on either engine).
  - x is held in bf16 end-to-end (update, life mask, store); host casts.
  - mm2 pixel-major: stationary = relu'd h tile [128d,128px], moving =
    w2 [128,16] -> dx lands [px,16] in PSUM; per-bank evac applies the
    update mask and x += dx*um on V (PSUM path) or S-copy + Pool.
  - living-mask pools run pixel-major on Pool; final x*life and the
    store are split in quarters across V/P and the gpsimd DMA ring.
Host does layout transforms only; HW exec time is what's measured.
"""

import numpy as np

# ---------------------------------------------------------------- constants
B, C, H, W = 16, 16, 256, 256
NCORES = 8
SPC = B // NCORES          # samples per core
HWPX = H * W               # 65536 pixels per sample
PITCH = 258                # padded row pitch (wrap col + 256 + wrap col)
NROWH = 34                 # rows -1..32 (halo top/bottom) for xbf
XBF_F = NROWH * PITCH      # 8772
SOB_F = 32 * PITCH         # 8256 (rows 0..31 padded) for A/V/D
PIX_F = 8192               # 512 tiles * 16 ch  (xt free dim)
NT = HWPX // 128           # 512 pixel-tiles per sample
NSTRIP = 8                 # strips of 32 rows (8192 px each)
SPPX = 8192                # pixels per strip
KROWS = 97                 # S rows: 16x + 32V + 48D + ones
ALPHA_TH = 0.1
FIRE = 0.5

# chunks per strip: (col offset, width) within the strip's 8192 px
CHUNKS = [(0, 1536), (1536, 1536), (3072, 1536), (4608, 1536),
          (6144, 1536), (7680, 512)]

# ------------------------------------------------------------ tuning knobs
EVAC_PAT = "SVSSVS"   # relu-evac engine per chunk (rotating): S=scalar, V=vector
BANK_PAT = "VP"       # dx-bank evac per bank: V=vector, P=scalar-copy+pool
MULT_PAT = "VPPP"     # final x*life quarters
SOB_A = "P"           # A = up+dn
SOB_D = "V"           # D = dn-up
POOL_BIG = "V"        # big ops of the 3x3 max pool (max: DVE-only)
LIFE_ENG = "P"        # life = preM*postM

_BUILT = None


# ------------------------------------------------------------- host layouts
def _bf16():
    import ml_dtypes
    return ml_dtypes.bfloat16


def _prep_xbf(x):
    """x: [B, C, H, W] f32 -> [B, 128, XBF_F] bf16 strip layout w/ halo+wrap.

    partition p = hb*16 + c ; free = (r, pc): r = hl+1 for hl in -1..32,
    pc: 0 <-> w=255, 1..256 <-> w=0..255, 257 <-> w=0.   h = hb*32 + hl mod 256
    """
    bf16 = _bf16()
    xb = x.astype(bf16)                                   # [B, C, H, W]
    hidx = (np.arange(-1, 33)[None, :] + 32 * np.arange(8)[:, None]) % 256
    xr = xb[:, :, hidx, :]                                # [B, C, 8, 34, W]
    out = np.empty((B, 8, C, NROWH, PITCH), dtype=bf16)
    out[:, :, :, :, 1:257] = np.transpose(xr, (0, 2, 1, 3, 4))
    out[:, :, :, :, 0] = np.transpose(xr[:, :, :, :, 255], (0, 2, 1, 3))
    out[:, :, :, :, 257] = np.transpose(xr[:, :, :, :, 0], (0, 2, 1, 3))
    return np.ascontiguousarray(out.reshape(B, 128, XBF_F))


def _prep_xt(x):
    """x: [B, C, H, W] f32 -> pixel-major [B, 128, 8192] bf16.

    xt[b, p, 16*t + c] = x[b, c, pix] with pix = 128*t + p (raster order).
    """
    bf16 = _bf16()
    xf = x.reshape(B, C, HWPX).transpose(0, 2, 1)         # [B, pix, C]
    xf = xf.reshape(B, NT, 128, C).transpose(0, 2, 1, 3)  # [B, p, t, c]
    return np.ascontiguousarray(xf.reshape(B, 128, NT * C).astype(bf16))


def _prep_xcm(x):
    """x [B,C,H,W] f32 -> [B, 8, 16, 8192] bf16: per-strip channel-major."""
    bf16 = _bf16()
    xs = x.reshape(B, C, NSTRIP, 32 * W).transpose(0, 2, 1, 3)
    return np.ascontiguousarray(xs.astype(bf16))


def _prep_randt(rv):
    """rand_vals [B, 1, H, W] -> [B, 128, NT] f32, rt[b, p, t] = rv[b, pix]."""
    rf = rv.reshape(B, HWPX).reshape(B, NT, 128).transpose(0, 2, 1)
    return np.ascontiguousarray(rf.astype(np.float32))


def _unprep_out(op):
    """out_pm [B, 128, 8192] bf16 -> [B, C, H, W] f32."""
    o = op.astype(np.float32).reshape(B, 128, NT, C).transpose(0, 2, 1, 3)
    o = o.reshape(B, HWPX, C).transpose(0, 2, 1)
    return np.ascontiguousarray(o.reshape(B, C, H, W))


def _prep_weights(w1, b1, w2, b2):
    bf16 = _bf16()
    w1 = np.asarray(w1, np.float32)
    w2 = np.asarray(w2, np.float32)
    wid, wdx, wdy = w1[0::3], w1[1::3], w1[2::3]
    # S rows: [x; V(w-1); V(w+1); D(w-1); D(w); D(w+1); ones]
    # pdx = 0.125*(V(w+1)-V(w-1)) ; pdy = 0.125*(D(w-1)+2D(w)+D(w+1))
    w1e = np.concatenate([
        wid,
        -0.125 * wdx,
        0.125 * wdx,
        0.125 * wdy,
        0.25 * wdy,
        0.125 * wdy,
        np.asarray(b1, np.float32).reshape(1, 128),
    ], axis=0)                                            # [97, 128]
    return (np.ascontiguousarray(w1e.astype(bf16)),
            np.ascontiguousarray(w2.astype(bf16)),
            np.asarray(b2, np.float32).reshape(1, 16))


# ------------------------------------------------------------- build module
def _build(b2_nonzero):
    import concourse.bass as bass
    import concourse.bacc as bacc
    import concourse.mybir as mybir
    import concourse.tile as tile

    dt = mybir.dt
    op = mybir.AluOpType
    AF = mybir.ActivationFunctionType

    nc = bacc.Bacc("TRN2", target_bir_lowering=False, debug=False)

    xbf_d = nc.dram_tensor("xbf", (SPC, 128, XBF_F), dt.bfloat16, kind="ExternalInput")
    xcm_d = nc.dram_tensor("xcm", (SPC, NSTRIP, 16, SPPX), dt.bfloat16, kind="ExternalInput")
    xt_d = nc.dram_tensor("xt", (SPC, 128, PIX_F), dt.bfloat16, kind="ExternalInput")
    rt_d = nc.dram_tensor("rt", (SPC, 128, NT), dt.float32, kind="ExternalInput")
    w1_d = nc.dram_tensor("w1e", (KROWS, 128), dt.bfloat16, kind="ExternalInput")
    w2_d = nc.dram_tensor("w2e", (128, 16), dt.bfloat16, kind="ExternalInput")
    ones_d = nc.dram_tensor("onesr", (1, SPPX), dt.bfloat16, kind="ExternalInput")
    b2_d = nc.dram_tensor("b2e", (1, 16), dt.float32, kind="ExternalInput")
    out_d = nc.dram_tensor("outp", (SPC, 128, PIX_F), dt.bfloat16, kind="ExternalOutput")

    def eng(name):
        return {"V": nc.vector, "P": nc.gpsimd}[name]

    with tile.TileContext(nc) as tc:
        with (
            tc.tile_pool(name="wpool", bufs=1) as wpool,
            tc.tile_pool(name="xbf", bufs=1) as p_xbf,
            tc.tile_pool(name="pA", bufs=1) as p_A,
            tc.tile_pool(name="pV", bufs=2) as p_V,
            tc.tile_pool(name="pD", bufs=2) as p_D,
            tc.tile_pool(name="xt", bufs=2) as p_xt,
            tc.tile_pool(name="S", bufs=2) as p_S,
            tc.tile_pool(name="hsb", bufs=2) as p_hsb,
            tc.tile_pool(name="small", bufs=2) as p_small,
            tc.tile_pool(name="dx", bufs=2) as p_dx,
            tc.tile_pool(name="pscr", bufs=2) as p_pscr,
            tc.tile_pool(name="psh", bufs=2, space=bass.MemorySpace.PSUM) as p_psh,
            tc.tile_pool(name="psdx", bufs=2, space=bass.MemorySpace.PSUM) as p_psdx,
        ):
            w1_sb = wpool.tile([KROWS, 128], dt.bfloat16, tag="w1")
            nc.sync.dma_start(w1_sb[:], w1_d.ap())
            w2_sb = wpool.tile([128, 16], dt.bfloat16, tag="w2")
            nc.sync.dma_start(w2_sb[:], w2_d.ap())
            if b2_nonzero:
                b2_sb = wpool.tile([128, 16], dt.float32, tag="b2")
                nc.sync.dma_start(b2_sb[:], b2_d.ap().broadcast_to([128, 16]))

            # prime the ones row (row 96) of both S slots; the per-strip
            # gathers only write rows 0..95, so it persists across reuse.
            s_prime = []
            for _ in range(2):
                St = p_S.tile([KROWS, SPPX], dt.bfloat16, tag="S")
                nc.sync.dma_start(St[96:97, :], ones_d.ap())
                s_prime.append(St)

            def emit_head(s):
                st = {}
                xbf = p_xbf.tile([128, XBF_F], dt.bfloat16, tag="xbf")
                nc.scalar.dma_start(xbf[:], xbf_d.ap()[s])
                xt = p_xt.tile([128, PIX_F], dt.bfloat16, tag="xt")
                nc.scalar.dma_start(xt[:], xt_d.ap()[s])
                rt = p_small.tile([128, NT], dt.float32, tag="rt")
                nc.scalar.dma_start(rt[:], rt_d.ap()[s])

                xbf3 = xbf.rearrange("p (r q) -> p r q", q=PITCH)
                x_up = xbf3[:, 0:32, :]
                x_mid = xbf3[:, 1:33, :]
                x_dn = xbf3[:, 2:34, :]
                A = p_A.tile([128, SOB_F], dt.bfloat16, tag="A")
                A3 = A.rearrange("p (r q) -> p r q", q=PITCH)
                Vt = p_V.tile([128, SOB_F], dt.bfloat16, tag="V")
                V3 = Vt.rearrange("p (r q) -> p r q", q=PITCH)
                Dt = p_D.tile([128, SOB_F], dt.bfloat16, tag="D")
                D3 = Dt.rearrange("p (r q) -> p r q", q=PITCH)
                eng(SOB_A).tensor_tensor(A3[:], x_up, x_dn, op.add)
                # V = 2*mid + A  (scalar_tensor_tensor, 1x)
                nc.vector.scalar_tensor_tensor(
                    V3[:], x_mid, 2.0, A3[:], op.mult, op.add)
                eng(SOB_D).tensor_tensor(D3[:], x_dn, x_up, op.subtract)

                um = p_small.tile([128, NT], dt.bfloat16, tag="um")
                nc.vector.tensor_scalar(um[:], rt[:], FIRE, None, op.is_lt)
                st.update(xt=xt, xt3=xt.rearrange("p (t c) -> p t c", c=16),
                          V=Vt, D=Dt, um=um)
                return st

            def emit_head2(s, st):
                alP = p_small.tile([128, NT], dt.bfloat16, tag="alP")
                nc.vector.tensor_copy(alP[:], st["xt3"][:, :, 3])
                preM = p_small.tile([128, NT], dt.bfloat16, tag="preM")
                _pool_and_thresh(nc, p_pscr, alP, preM, op, dt, eng(POOL_BIG))
                st["preM"] = preM

            def emit_mid(s, st, strips, counters):
                xt, um = st["xt"], st["um"]
                V3 = st["V"].rearrange("p (r q) -> p r q", q=PITCH)
                D3 = st["D"].rearrange("p (r q) -> p r q", q=PITCH)
                for hb in strips:
                    S = p_S.tile([KROWS, SPPX], dt.bfloat16, tag="S")
                    S4 = S.rearrange("k (r c) -> k r c", c=W)
                    pp = slice(16 * hb, 16 * hb + 16)
                    # gathers (rows 0..95); row 96 = ones persists
                    nc.scalar.dma_start(S[0:16, :], xcm_d.ap()[s, hb])
                    nc.sync.dma_start(S4[16:32], V3[pp, :, 0:256])    # V(w-1)
                    nc.sync.dma_start(S4[32:48], V3[pp, :, 2:258])    # V(w+1)
                    nc.scalar.dma_start(S4[48:64], D3[pp, :, 0:256])  # D(w-1)
                    nc.sync.dma_start(S4[64:80], D3[pp, :, 1:257])    # D(w)
                    nc.scalar.dma_start(S4[80:96], D3[pp, :, 2:258])  # D(w+1)

                    psdx = None
                    for ci, (c0, cw) in enumerate(CHUNKS):
                        psh = p_psh.tile([128, 1536], dt.float32, tag="psh")
                        for j in range(cw // 512):
                            nc.tensor.matmul(
                                psh[:, 512 * j:512 * (j + 1)],
                                w1_sb[:],
                                S[:, c0 + 512 * j:c0 + 512 * (j + 1)])
                        hsb = p_hsb.tile([128, 1536], dt.bfloat16, tag="hsb")
                        e = EVAC_PAT[counters["chunk"] % len(EVAC_PAT)]
                        counters["chunk"] += 1
                        if e == "S":
                            nc.scalar.activation(hsb[:, :cw], psh[:, :cw], AF.Relu)
                        else:
                            nc.vector.tensor_scalar(
                                hsb[:, :cw], psh[:, :cw], 0.0, None, op.max)
                        # mm2 for this chunk's tiles
                        t0 = c0 // 128
                        for tl in range(cw // 128):
                            t = t0 + tl            # tile within strip 0..63
                            if t % 32 == 0:
                                psdx = p_psdx.tile([128, 512], dt.float32,
                                                   tag="psdx")
                                st["bank%d" % (2 * hb + t // 32)] = psdx
                            nc.tensor.matmul(
                                psdx[:, 16 * (t % 32):16 * (t % 32) + 16],
                                hsb[:, 128 * tl:128 * (tl + 1)],
                                w2_sb[:])
                        if ci == 2:
                            _evac_bank(nc, st, 2 * hb, um, xt, op, dt, p_dx,
                                       counters, b2_sb if b2_nonzero else None)
                    _evac_bank(nc, st, 2 * hb + 1, um, xt, op, dt, p_dx,
                               counters, b2_sb if b2_nonzero else None)

            def emit_tail(s, st):
                xt, xt3 = st["xt"], st["xt3"]
                alN = p_small.tile([128, NT], dt.bfloat16, tag="alN")
                nc.vector.tensor_copy(alN[:], xt3[:, :, 3])
                postM = p_small.tile([128, NT], dt.bfloat16, tag="postM")
                _pool_and_thresh(nc, p_pscr, alN, postM, op, dt, eng(POOL_BIG))
                life = p_small.tile([128, NT], dt.bfloat16, tag="life")
                eng(LIFE_ENG).tensor_tensor(life[:], st["preM"][:], postM[:],
                                            op.mult)
                for q in range(4):
                    tq = slice(128 * q, 128 * (q + 1))
                    cq = slice(2048 * q, 2048 * (q + 1))
                    eng(MULT_PAT[q]).tensor_tensor(
                        xt3[:, tq, :], xt3[:, tq, :],
                        life[:, tq].broadcast_to([128, 128, 16]), op.mult)
                    nc.gpsimd.dma_start(out_d.ap()[s][:, cq], xt[:, cq])

            counters = {"chunk": 0, "bank": 0}
            st0 = emit_head(0)
            emit_head2(0, st0)
            emit_mid(0, st0, range(0, 4), counters)
            st1 = emit_head(1)
            emit_mid(0, st0, range(4, 8), counters)
            emit_head2(1, st1)
            emit_tail(0, st0)
            emit_mid(1, st1, range(0, 8), counters)
            emit_tail(1, st1)

    nc.compile()
    return nc


def _evac_bank(nc, st, k, um, xt, op, dt, p_dx, counters, b2_sb):
    """Bank k (4096 px): apply update mask and x += dx*um."""
    psdx = st.pop("bank%d" % k)
    ps3 = psdx.rearrange("p (t c) -> p t c", c=16)
    umk = um[:, 32 * k:32 * k + 32]
    sl = slice(512 * k, 512 * (k + 1))
    mode = BANK_PAT[counters["bank"] % len(BANK_PAT)]
    counters["bank"] += 1
    if b2_sb is not None:
        nc.vector.tensor_tensor(
            ps3[:], ps3[:],
            b2_sb[:].rearrange("p c -> p 1 c").broadcast_to([128, 32, 16]),
            op.add)
    if mode == "V":
        DXM = p_dx.tile([128, 512], dt.bfloat16, tag="DXM")
        nc.vector.tensor_tensor(
            DXM.rearrange("p (t c) -> p t c", c=16), ps3[:],
            umk.broadcast_to([128, 32, 16]), op.mult)
        nc.vector.tensor_tensor(xt[:, sl], xt[:, sl], DXM[:], op.add)
    else:
        DXE = p_dx.tile([128, 512], dt.bfloat16, tag="DXE")
        nc.scalar.copy(DXE[:], psdx[:])
        DXM = p_dx.tile([128, 512], dt.bfloat16, tag="DXM")
        nc.gpsimd.tensor_tensor(
            DXM.rearrange("p (t c) -> p t c", c=16),
            DXE.rearrange("p (t c) -> p t c", c=16),
            umk.broadcast_to([128, 32, 16]), op.mult)
        nc.gpsimd.tensor_tensor(xt[:, sl], xt[:, sl], DXM[:], op.add)


def _pool_and_thresh(nc, pool, alpha, outM, op, dt, big):
    """3x3 circular max-pool on pixel-major alpha [128, NT] then > ALPHA_TH.

    pix = 128*t + p ;  w-neighbors: partition +-1 ; h-neighbors: t -+ 2.
    Partition-shifted neighbor tensors (aL/aR) staged via SBUF->SBUF DMAs.
    """
    f16 = dt.bfloat16
    aL = pool.tile([128, NT], f16, tag="aL")
    aR = pool.tile([128, NT], f16, tag="aR")
    nc.sync.dma_start(aL[1:128, :], alpha[0:127, :])
    nc.sync.dma_start(aR[0:127, :], alpha[1:128, :])
    eL = pool.tile([1, NT], f16, tag="eL")
    nc.sync.dma_start(eL[:], alpha[127:128, :])
    nc.vector.tensor_copy(aL[0:1, 0:NT:2], eL[0:1, 1:NT:2])
    nc.vector.tensor_copy(aL[0:1, 1:NT:2], eL[0:1, 0:NT - 1:2])
    edr = pool.tile([1, NT], f16, tag="edr")
    nc.vector.tensor_copy(edr[0:1, 0:NT:2], alpha[0:1, 1:NT:2])
    nc.vector.tensor_copy(edr[0:1, 1:NT:2], alpha[0:1, 0:NT - 1:2])
    nc.sync.dma_start(aR[127:128, :], edr[:])
    PW = pool.tile([128, NT], f16, tag="PW")
    big.tensor_tensor(PW[:], alpha[:, :], aL[:], op.max)
    big.tensor_tensor(PW[:], PW[:], aR[:], op.max)
    z2 = pool.tile([128, NT], f16, tag="z2")
    big.tensor_tensor(z2[:, 0:NT - 2], PW[:, 0:NT - 2], PW[:, 2:NT], op.max)
    big.tensor_tensor(outM[:, 2:NT - 2], z2[:, 0:NT - 4], PW[:, 4:NT], op.max)
    nc.vector.tensor_tensor(outM[:, 0:2], z2[:, 0:2], PW[:, NT - 2:NT], op.max)
    nc.vector.tensor_tensor(outM[:, NT - 2:NT], z2[:, NT - 4:NT - 2],
                            PW[:, 0:2], op.max)
    nc.vector.tensor_scalar(outM[:], outM[:], ALPHA_TH, None, op.is_gt)


def _get_built(b2_nonzero):
    global _BUILT
    if _BUILT is None or _BUILT[0] != b2_nonzero:
        _BUILT = (b2_nonzero, _build(b2_nonzero))
    return _BUILT[1]


# ------------------------------------------------------------------ kernel
def kernel(x, rand_vals, w1, b1, w2, b2):
    from concourse.bass_utils import run_bass_kernel_spmd

    bf16 = _bf16()
    x = np.asarray(x, np.float32)
    rand_vals = np.asarray(rand_vals, np.float32)
    w1e, w2e, b2e = _prep_weights(w1, b1, w2, b2)
    b2_nonzero = bool(np.any(b2e != 0.0))

    xbf = _prep_xbf(x)
    xt = _prep_xt(x)
    xcm = _prep_xcm(x)
    rt = _prep_randt(rand_vals)
    ones = np.ones((1, SPPX), dtype=bf16)

    nc = _get_built(b2_nonzero)

    in_maps = []
    for i in range(NCORES):
        sl = slice(SPC * i, SPC * (i + 1))
        in_maps.append({
            "xbf": np.ascontiguousarray(xbf[sl]),
            "xcm": np.ascontiguousarray(xcm[sl]),
            "xt": np.ascontiguousarray(xt[sl]),
            "rt": np.ascontiguousarray(rt[sl]),
            "w1e": w1e, "w2e": w2e, "onesr": ones,
            "b2e": b2e.reshape(1, 16),
        })

    res = run_bass_kernel_spmd(nc, in_maps, core_ids=list(range(NCORES)))
    outs = [res.results[i]["outp"] for i in range(NCORES)]
    out_pm = np.concatenate(outs, axis=0)        # [B, 128, 8192] bf16
    return _unprep_out(out_pm)


# revision 5
# speedup vs baseline: 1.0645x; 1.0645x over previous
"""Trainium2 Bass kernel for nn_CAModel (neural cellular automaton step).

v3 strategy (data-parallel over batch, 16 samples -> 8 cores x 2):
  - The w-direction sobel taps are folded into mm1's contraction dim:
    S rows = [x(16); V(w-1)(16); V(w+1)(16); D(w-1); D(w); D(w+1); ones]
    with V = [1,2,1]_h smoothing and D = x(h+1)-x(h-1).  K=97.
  - The S staging tile lives in PADDED pixel space (pitch 258, like the
    sobel tensors), so every shift gather is one CONTIGUOUS SBUF DMA;
    the padded sobel outputs already hold correct wrap columns.  mm1
    streams 0.8% extra (pad) columns; pixel tiles of 128 never straddle
    a row (256 = 2*128), just offset by (t//2)*258 + 1 + (t%2)*128.
  - ones row carries b1 into mm1, so relu evac = max(psum,0): split
    between ScalarE ACTIVATE and VectorE TENSOR_SCALAR by pattern.
  - x held in bf16 end-to-end; host casts/layouts.
  - Per-strip hsb [128,8256] and psdx [128,1024] (64 tiles * 16ch);
    bank evac (mask + x update) on Vector.
  - 3x3 living-mask pool: neighbor staging entirely by DMA (no V<->DMA
    ping-pong), then one straight-line Vector block.
Host does layout transforms only; HW exec time is what's measured.
"""

import numpy as np

# ---------------------------------------------------------------- constants
B, C, H, W = 16, 16, 256, 256
NCORES = 8
SPC = B // NCORES          # samples per core
HWPX = H * W               # 65536 pixels per sample
PITCH = 258                # padded row pitch (wrap col + 256 + wrap col)
NROWH = 34                 # rows -1..32 (halo top/bottom) for xbf
XBF_F = NROWH * PITCH      # 8772
SOB_F = 32 * PITCH         # 8256 (rows 0..31 padded) for A/V/D and S
PIX_F = 8192               # 512 tiles * 16 ch  (xt free dim)
NT = HWPX // 128           # 512 pixel-tiles per sample
NSTRIP = 8                 # strips of 32 rows (8192 px each)
KROWS = 97                 # S rows: 16x + 32V + 48D + ones
ALPHA_TH = 0.1
FIRE = 0.5

# psh chunks per strip: (col offset, width) in padded strip space (8256)
CHUNKS = [(0, 1536), (1536, 1536), (3072, 1536), (4608, 1536),
          (6144, 1536), (7680, 576)]

# ------------------------------------------------------------ tuning knobs
EVAC_V_EVERY = 6      # every Nth chunk's relu evac goes to Vector (rest S)
MULT_PAT = "VP"       # final x*life halves
LIFE_ENG = "P"

_BUILT = None


# ------------------------------------------------------------- host layouts
def _bf16():
    import ml_dtypes
    return ml_dtypes.bfloat16


def _pad_wrap(a):
    """a: [..., 256] -> [..., 258] with wrap cols."""
    out = np.empty(a.shape[:-1] + (PITCH,), dtype=a.dtype)
    out[..., 1:257] = a
    out[..., 0] = a[..., 255]
    out[..., 257] = a[..., 0]
    return out


def _prep_xbf(x):
    """x: [B, C, H, W] f32 -> [B, 128, XBF_F] bf16 strip layout w/ halo+wrap.

    partition p = hb*16 + c ; free = (r, pc): r = hl+1 for hl in -1..32,
    pc: 0 <-> w=255, 1..256 <-> w=0..255, 257 <-> w=0.
    """
    bf16 = _bf16()
    xb = x.astype(bf16)
    hidx = (np.arange(-1, 33)[None, :] + 32 * np.arange(8)[:, None]) % 256
    xr = xb[:, :, hidx, :]                                # [B, C, 8, 34, W]
    out = _pad_wrap(np.transpose(xr, (0, 2, 1, 3, 4)))    # [B,8,C,34,258]
    return np.ascontiguousarray(out.reshape(B, 128, XBF_F))


def _prep_xt(x):
    """x: [B, C, H, W] f32 -> pixel-major [B, 128, 8192] bf16."""
    bf16 = _bf16()
    xf = x.reshape(B, C, HWPX).transpose(0, 2, 1)
    xf = xf.reshape(B, NT, 128, C).transpose(0, 2, 1, 3)
    return np.ascontiguousarray(xf.reshape(B, 128, NT * C).astype(bf16))


def _prep_xcm(x):
    """x [B,C,H,W] f32 -> [B, 8, 16, SOB_F] bf16: per-strip channel-major,
    rows padded to pitch 258 with wrap cols."""
    bf16 = _bf16()
    xs = x.reshape(B, C, NSTRIP, 32, W).transpose(0, 2, 1, 3, 4)
    xp = _pad_wrap(xs.astype(bf16))                        # [B,8,C,32,258]
    return np.ascontiguousarray(xp.reshape(B, NSTRIP, 16, SOB_F))


def _prep_randt(rv):
    rf = rv.reshape(B, HWPX).reshape(B, NT, 128).transpose(0, 2, 1)
    return np.ascontiguousarray(rf.astype(np.float32))


def _unprep_out(op):
    o = op.astype(np.float32).reshape(B, 128, NT, C).transpose(0, 2, 1, 3)
    o = o.reshape(B, HWPX, C).transpose(0, 2, 1)
    return np.ascontiguousarray(o.reshape(B, C, H, W))


def _prep_weights(w1, b1, w2, b2):
    bf16 = _bf16()
    w1 = np.asarray(w1, np.float32)
    w2 = np.asarray(w2, np.float32)
    wid, wdx, wdy = w1[0::3], w1[1::3], w1[2::3]
    w1e = np.concatenate([
        wid,
        -0.125 * wdx,          # V(w-1)
        0.125 * wdx,           # V(w+1)
        0.125 * wdy,           # D(w-1)
        0.25 * wdy,            # D(w)
        0.125 * wdy,           # D(w+1)
        np.asarray(b1, np.float32).reshape(1, 128),
    ], axis=0)                                            # [97, 128]
    return (np.ascontiguousarray(w1e.astype(bf16)),
            np.ascontiguousarray(w2.astype(bf16)),
            np.asarray(b2, np.float32).reshape(1, 16))


# ------------------------------------------------------------- build module
def _build(b2_nonzero):
    import concourse.bass as bass
    import concourse.bacc as bacc
    import concourse.mybir as mybir
    import concourse.tile as tile

    dt = mybir.dt
    op = mybir.AluOpType
    AF = mybir.ActivationFunctionType

    nc = bacc.Bacc("TRN2", target_bir_lowering=False, debug=False)

    xbf_d = nc.dram_tensor("xbf", (SPC, 128, XBF_F), dt.bfloat16, kind="ExternalInput")
    xcm_d = nc.dram_tensor("xcm", (SPC, NSTRIP, 16, SOB_F), dt.bfloat16, kind="ExternalInput")
    xt_d = nc.dram_tensor("xt", (SPC, 128, PIX_F), dt.bfloat16, kind="ExternalInput")
    rt_d = nc.dram_tensor("rt", (SPC, 128, NT), dt.float32, kind="ExternalInput")
    w1_d = nc.dram_tensor("w1e", (KROWS, 128), dt.bfloat16, kind="ExternalInput")
    w2_d = nc.dram_tensor("w2e", (128, 16), dt.bfloat16, kind="ExternalInput")
    ones_d = nc.dram_tensor("onesr", (1, SOB_F), dt.bfloat16, kind="ExternalInput")
    b2_d = nc.dram_tensor("b2e", (1, 16), dt.float32, kind="ExternalInput")
    out_d = nc.dram_tensor("outp", (SPC, 128, PIX_F), dt.bfloat16, kind="ExternalOutput")

    def eng(name):
        return {"V": nc.vector, "P": nc.gpsimd}[name]

    with tile.TileContext(nc) as tc:
        with (
            tc.tile_pool(name="wpool", bufs=1) as wpool,
            tc.tile_pool(name="xbf", bufs=1) as p_xbf,
            tc.tile_pool(name="pA", bufs=1) as p_A,
            tc.tile_pool(name="pV", bufs=1) as p_V,
            tc.tile_pool(name="pD", bufs=1) as p_D,
            tc.tile_pool(name="xt", bufs=2) as p_xt,
            tc.tile_pool(name="S", bufs=2) as p_S,
            tc.tile_pool(name="hsb", bufs=2) as p_hsb,
            tc.tile_pool(name="small", bufs=2) as p_small,
            tc.tile_pool(name="dx", bufs=2) as p_dx,
            tc.tile_pool(name="pscr", bufs=2) as p_pscr,
            tc.tile_pool(name="psh", bufs=2, space=bass.MemorySpace.PSUM) as p_psh,
            tc.tile_pool(name="psdx", bufs=1, space=bass.MemorySpace.PSUM) as p_psdx,
        ):
            w1_sb = wpool.tile([KROWS, 128], dt.bfloat16, tag="w1")
            nc.sync.dma_start(w1_sb[:], w1_d.ap())
            w2_sb = wpool.tile([128, 16], dt.bfloat16, tag="w2")
            nc.sync.dma_start(w2_sb[:], w2_d.ap())
            if b2_nonzero:
                b2_sb = wpool.tile([128, 16], dt.float32, tag="b2")
                nc.sync.dma_start(b2_sb[:], b2_d.ap().broadcast_to([128, 16]))

            # prime both S slots: ones row (row 96) + edge cols that the
            # contiguous shift gathers never write.  All persist: per-strip
            # gathers only write rows 0..95 / the interior cols.
            for _ in range(2):
                St = p_S.tile([KROWS, SOB_F], dt.bfloat16, tag="S")
                nc.sync.dma_start(St[96:97, :], ones_d.ap())
                nc.vector.memset(St[:, 0:1], 0.0)
                nc.vector.memset(St[:, SOB_F - 1:SOB_F], 0.0)

            def emit_head(s):
                st = {}
                xbf = p_xbf.tile([128, XBF_F], dt.bfloat16, tag="xbf")
                nc.scalar.dma_start(xbf[:], xbf_d.ap()[s])
                xt = p_xt.tile([128, PIX_F], dt.bfloat16, tag="xt")
                nc.scalar.dma_start(xt[:], xt_d.ap()[s])
                rt = p_small.tile([128, NT], dt.float32, tag="rt")
                nc.scalar.dma_start(rt[:], rt_d.ap()[s])

                xbf3 = xbf.rearrange("p (r q) -> p r q", q=PITCH)
                x_up = xbf3[:, 0:32, :]
                x_mid = xbf3[:, 1:33, :]
                x_dn = xbf3[:, 2:34, :]
                A = p_A.tile([128, SOB_F], dt.bfloat16, tag="A")
                Vt = p_V.tile([128, SOB_F], dt.bfloat16, tag="V")
                Dt = p_D.tile([128, SOB_F], dt.bfloat16, tag="D")
                A3 = A.rearrange("p (r q) -> p r q", q=PITCH)
                V3 = Vt.rearrange("p (r q) -> p r q", q=PITCH)
                D3 = Dt.rearrange("p (r q) -> p r q", q=PITCH)
                nc.vector.tensor_tensor(A3[:], x_up, x_dn, op.add)
                nc.vector.scalar_tensor_tensor(
                    V3[:], x_mid, 2.0, A3[:], op.mult, op.add)
                nc.vector.tensor_tensor(D3[:], x_dn, x_up, op.subtract)

                um = p_small.tile([128, NT], dt.bfloat16, tag="um")
                nc.vector.tensor_scalar(um[:], rt[:], FIRE, None, op.is_lt)
                st.update(xt=xt, xt3=xt.rearrange("p (t c) -> p t c", c=16),
                          V=Vt, D=Dt, um=um)
                return st

            def emit_head2(s, st):
                alP = p_small.tile([128, NT], dt.bfloat16, tag="alP")
                nc.vector.tensor_copy(alP[:], st["xt3"][:, :, 3])
                preM = p_small.tile([128, NT], dt.bfloat16, tag="preM")
                _pool_and_thresh(nc, p_pscr, alP, preM, op, dt)
                st["preM"] = preM

            def emit_mid(s, st, strips, counters):
                xt, um = st["xt"], st["um"]
                Vf, Df = st["V"], st["D"]
                for hb in strips:
                    S = p_S.tile([KROWS, SOB_F], dt.bfloat16, tag="S")
                    pp = slice(16 * hb, 16 * hb + 16)
                    F = SOB_F
                    # contiguous shift gathers; wrap cols are already
                    # correct in the padded sobel tensors.
                    nc.scalar.dma_start(S[0:16, :], xcm_d.ap()[s, hb])
                    nc.sync.dma_start(S[16:32, 1:F], Vf[pp, 0:F - 1])   # V(w-1)
                    nc.sync.dma_start(S[32:48, 0:F - 1], Vf[pp, 1:F])   # V(w+1)
                    nc.gpsimd.dma_start(S[48:64, 1:F], Df[pp, 0:F - 1]) # D(w-1)
                    nc.sync.dma_start(S[64:80, :], Df[pp, :])           # D(w)
                    nc.gpsimd.dma_start(S[80:96, 0:F - 1], Df[pp, 1:F]) # D(w+1)

                    hsb = p_hsb.tile([128, SOB_F], dt.bfloat16, tag="hsb")
                    psdx = p_psdx.tile([128, 1024], dt.float32, tag="psdx")
                    for (c0, cw) in CHUNKS:
                        psh = p_psh.tile([128, 1536], dt.float32, tag="psh")
                        nmm = (cw + 511) // 512
                        for j in range(nmm):
                            w = min(512, cw - 512 * j)
                            nc.tensor.matmul(
                                psh[:, 512 * j:512 * j + w],
                                w1_sb[:],
                                S[:, c0 + 512 * j:c0 + 512 * j + w])
                        ci = counters["chunk"]
                        counters["chunk"] += 1
                        if ci % EVAC_V_EVERY == EVAC_V_EVERY - 1:
                            nc.vector.tensor_scalar(
                                hsb[:, c0:c0 + cw], psh[:, :cw], 0.0, None,
                                op.max)
                        else:
                            nc.scalar.activation(
                                hsb[:, c0:c0 + cw], psh[:, :cw], AF.Relu)
                        # mm2 for the pixel tiles fully inside this chunk
                        t_lo = counters["t_next"]
                        while True:
                            t = counters["t_next"]
                            if t >= 64:
                                break
                            off = (t // 2) * PITCH + 1 + (t % 2) * 128
                            if off + 128 > c0 + cw:
                                break
                            nc.tensor.matmul(
                                psdx[:, 16 * (t % 64):16 * (t % 64) + 16],
                                hsb[:, off:off + 128],
                                w2_sb[:])
                            counters["t_next"] += 1
                    counters["t_next"] = 0
                    _evac_strip(nc, psdx, hb, um, xt, op, dt, p_dx,
                                b2_sb if b2_nonzero else None)

            def emit_tail(s, st):
                xt, xt3 = st["xt"], st["xt3"]
                alN = p_small.tile([128, NT], dt.bfloat16, tag="alN")
                nc.vector.tensor_copy(alN[:], xt3[:, :, 3])
                postM = p_small.tile([128, NT], dt.bfloat16, tag="postM")
                _pool_and_thresh(nc, p_pscr, alN, postM, op, dt)
                life = p_small.tile([128, NT], dt.bfloat16, tag="life")
                eng(LIFE_ENG).tensor_tensor(life[:], st["preM"][:], postM[:],
                                            op.mult)
                nh = len(MULT_PAT)
                tq = NT // nh
                for q in range(nh):
                    ts = slice(tq * q, tq * (q + 1))
                    cs = slice(16 * tq * q, 16 * tq * (q + 1))
                    eng(MULT_PAT[q]).tensor_tensor(
                        xt3[:, ts, :], xt3[:, ts, :],
                        life[:, ts].broadcast_to([128, tq, 16]), op.mult)
                    nc.gpsimd.dma_start(out_d.ap()[s][:, cs], xt[:, cs])

            counters = {"chunk": 0, "t_next": 0}
            st0 = emit_head(0)
            emit_head2(0, st0)
            emit_mid(0, st0, range(0, 4), counters)
            st1 = emit_head(1)
            emit_mid(0, st0, range(4, 8), counters)
            emit_head2(1, st1)
            emit_tail(0, st0)
            emit_mid(1, st1, range(0, 8), counters)
            emit_tail(1, st1)

    nc.compile()
    return nc


def _evac_strip(nc, psdx, hb, um, xt, op, dt, p_dx, b2_sb):
    """Strip hb (8192 px, 64 tiles): dx*um and x += in pixel-major."""
    ps3 = psdx.rearrange("p (t c) -> p t c", c=16)            # [128, 64, 16]
    umk = um[:, 64 * hb:64 * hb + 64]
    sl = slice(1024 * hb, 1024 * (hb + 1))
    if b2_sb is not None:
        nc.vector.tensor_tensor(
            ps3[:], ps3[:],
            b2_sb[:].rearrange("p c -> p 1 c").broadcast_to([128, 64, 16]),
            op.add)
    DXM = p_dx.tile([128, 1024], dt.bfloat16, tag="DXM")
    nc.vector.tensor_tensor(
        DXM.rearrange("p (t c) -> p t c", c=16), ps3[:],
        umk.broadcast_to([128, 64, 16]), op.mult)
    nc.vector.tensor_tensor(xt[:, sl], xt[:, sl], DXM[:], op.add)


def _pool_and_thresh(nc, pool, alpha, outM, op, dt):
    """3x3 circular max-pool on pixel-major alpha [128, NT] then > ALPHA_TH.

    pix = 128*t + p; w-neighbors: partition +-1; h-neighbors: t -+ 2.
    Neighbor tensors (aL/aR incl. the parity-swapped wrap rows) are staged
    entirely by DMA, then one straight-line Vector block.
    """
    f16 = dt.bfloat16
    aL = pool.tile([128, NT], f16, tag="aL")
    aR = pool.tile([128, NT], f16, tag="aR")
    nc.sync.dma_start(aL[1:128, :], alpha[0:127, :])
    nc.scalar.dma_start(aR[0:127, :], alpha[1:128, :])
    # wrap rows, parity-interleaved: left-of-p0 from alpha[127, t+-1],
    # right-of-p127 from alpha[0, t-+1]
    nc.sync.dma_start(aL[0:1, 0:NT:2], alpha[127:128, 1:NT:2])
    nc.sync.dma_start(aL[0:1, 1:NT:2], alpha[127:128, 0:NT - 1:2])
    nc.scalar.dma_start(aR[127:128, 0:NT:2], alpha[0:1, 1:NT:2])
    nc.scalar.dma_start(aR[127:128, 1:NT:2], alpha[0:1, 0:NT - 1:2])
    PW = pool.tile([128, NT], f16, tag="PW")
    nc.vector.tensor_tensor(PW[:], alpha[:, :], aL[:], op.max)
    nc.vector.tensor_tensor(PW[:], PW[:], aR[:], op.max)
    z2 = pool.tile([128, NT], f16, tag="z2")
    nc.vector.tensor_tensor(z2[:, 0:NT - 2], PW[:, 0:NT - 2], PW[:, 2:NT], op.max)
    nc.vector.tensor_tensor(outM[:, 2:NT - 2], z2[:, 0:NT - 4], PW[:, 4:NT], op.max)
    nc.vector.tensor_tensor(outM[:, 0:2], z2[:, 0:2], PW[:, NT - 2:NT], op.max)
    nc.vector.tensor_tensor(outM[:, NT - 2:NT], z2[:, NT - 4:NT - 2],
                            PW[:, 0:2], op.max)
    nc.vector.tensor_scalar(outM[:], outM[:], ALPHA_TH, None, op.is_gt)


def _get_built(b2_nonzero):
    global _BUILT
    if _BUILT is None or _BUILT[0] != b2_nonzero:
        _BUILT = (b2_nonzero, _build(b2_nonzero))
    return _BUILT[1]


# ------------------------------------------------------------------ kernel
def kernel(x, rand_vals, w1, b1, w2, b2):
    from concourse.bass_utils import run_bass_kernel_spmd

    bf16 = _bf16()
    x = np.asarray(x, np.float32)
    rand_vals = np.asarray(rand_vals, np.float32)
    w1e, w2e, b2e = _prep_weights(w1, b1, w2, b2)
    b2_nonzero = bool(np.any(b2e != 0.0))

    xbf = _prep_xbf(x)
    xt = _prep_xt(x)
    xcm = _prep_xcm(x)
    rt = _prep_randt(rand_vals)
    ones = np.ones((1, SOB_F), dtype=bf16)

    nc = _get_built(b2_nonzero)

    in_maps = []
    for i in range(NCORES):
        sl = slice(SPC * i, SPC * (i + 1))
        in_maps.append({
            "xbf": np.ascontiguousarray(xbf[sl]),
            "xcm": np.ascontiguousarray(xcm[sl]),
            "xt": np.ascontiguousarray(xt[sl]),
            "rt": np.ascontiguousarray(rt[sl]),
            "w1e": w1e, "w2e": w2e, "onesr": ones,
            "b2e": b2e.reshape(1, 16),
        })

    res = run_bass_kernel_spmd(nc, in_maps, core_ids=list(range(NCORES)))
    outs = [res.results[i]["outp"] for i in range(NCORES)]
    out_pm = np.concatenate(outs, axis=0)
    return _unprep_out(out_pm)
